# revision 46
# baseline (speedup 1.0000x reference)
"""Trainium2 Bass kernel for nn_LogicLayer (soft logic-gate layer).

Math (single core):
  pA = softmax(Wa, axis=1); pB = softmax(Wb, axis=1); pT = softmax(tw, axis=0)
  a = pA @ X ; b = pB @ X
  out = sum_g pT[g] * gate_g(a, b)

Each of the 16 soft gates is affine in {1, A, B, A*B}, so with C[g, :] =
(c1, cA, cB, cAB) per gate:
  out = w1 + wA*a + wB*b + wAB*(a*b),   w_j[m] = sum_g pT[g, m] * C[g, j]

All softmax normalizers fold into the coefficients: with unnormalized
Ea = exp(Wa) (no max-subtraction needed; Wa ~ N(0,1)), ta = Ea^T-matmul,
a = ta / sA[m], and pT = exp(tw)/sT:
  out = w1' + wA'*ta + wB'*tb + wAB'*ta*tb

Dispatch-dominated regime: on this axon-tunneled setup, per-execute cost is
  base (~2.4-6 ms, host-load dependent) + ~0.14 ms/operand + input-bytes
  marshalled per execute, while on-device exec is only ~0.33 ms. Hence:
ONE core, minimal operands (fp8 x [16 MiB] + fp16 weights [4 MiB]), no
zero-output staging operands.

On-device design (TimelineSim-verified, 914 us -> 334 us):
  - fp8e4m3 + DoubleRow matmuls: K=256 stationaries [128, 2, 128] at
    0.5 cyc/row -> PE engine 872 -> 221 us, and half the PE sequencer
    dispatches. Row sums are over the QUANTIZED exps, so the on-chip
    softmax is exact for the quantized logits (common-mode fp8 error
    cancels in the normalizer). Measured rel err 3.1e-3 (gate 2e-2).
  - Merged DMA instructions: one grouped-rearrange DMA per n-tile group
    for all 8 k-blocks of x ("(c p) n -> p c n") and one for all 8
    m-blocks of out -> HWDGE dispatch 331 -> 51 us.
  - psum_bufs=3: PE runs 3 m-blocks ahead of the epilogue chain
    (u -> w -> o), removing ~100 us of PSUM-recycle stall.
  - Epilogue u alternates DVE/ACT (both compute scale*in+bias),
    balancing DVE 340 -> 256 us against ACT 253 us; GPSIMD o-add
    (285 us) is the remaining engine cap.

Unexplored frontier (priced, not attempted): replace the o-add with a
gpsimd SWDGE accumulate-DMA (dma_start(accum_op=add)) of a v-tile into
out_d after the w-tile copy. Requires bf16 out (else HBM write traffic
doubles past the win), correct Tile WAW ordering of two DMA writers to
the same DRAM region, and unpriced Q7 desc-gen cost. Best case ~40 us
(334 -> ~295 us exec), below host-noise measurability. Gate before
attempting: verify Tile's overlapping-view hazard tracker (tile.py,
"same-name access tracker") covers DRAM-tensor views, not just SBUF
tiles — this kernel only ever writes disjoint out_d regions, so the
ordering path is unexercised; a race here is intermittent corruption
that a single passing correctness run cannot rule out.
"""

import sys

if "/opt/trn_rl_repo" not in sys.path:
    sys.path.insert(0, "/opt/trn_rl_repo")

import numpy as np

import concourse.bass as bass
import concourse.mybir as mybir
import concourse.tile as tile

SIZE = 1024
PREV = 1024
BATCH = 16384
NB = BATCH             # single core: full batch
NT = 512               # n-tile (one PSUM bank of f32)
N_NT = NB // NT        # 32
KB = PREV // 128       # 8 k-blocks
MB = SIZE // 128       # 8 m-blocks

F32 = mybir.dt.float32
F16 = mybir.dt.float16

# Packed input column offsets (fp16 [PREV, PKW])
XC0 = 0
WA0 = BATCH
WB0 = WA0 + SIZE
TW0 = WB0 + SIZE
PKW = TW0 + 16

# Quantize x to uint8 on host: shrinks the dominant marshal operand from
# 32 MiB fp16 to 16 MiB. Dequant on-chip via ACT copy with scale+bias.
USE_U8_X = False

# fp8e4m3 + DoubleRow main matmuls: 2 fp8 k-rows per PE cell -> K=256
# stationaries at 0.5 cyc/row (4x PE time vs fp16) and half the PE
# sequencer dispatches. Row sums are taken over the QUANTIZED exps, so
# softmax normalization cancels fp8 common-mode error; x also fp8 (halves
# the marshal operand to 16 MiB).
USE_FP8_DR = True
F8 = mybir.dt.float8e4

# Merge the fp8 x and fp16 weight operands into ONE u8 tensor (bitcast
# per-region on device). Tested: outputs bit-identical, but paired A/B
# showed no speedup at 1 core (-110 us, within noise) -- the per-operand
# dispatch charge seen on the 8-core path does not materialize here.
# Keep False (the validated 2.90 ms configuration).
PACK1 = False
WBYTES = 2 * (2 * SIZE + 16)  # fp16 weight region in bytes

# Gate coefficient matrix: columns = [const, A, B, AB, ones]; rows = gate id.
_C16 = np.array(
    [
        # 1   A   B  AB  ones
        [0,  0,  0,  0, 1],  # 0  FALSE
        [0,  0,  0,  1, 1],  # 1  A AND B
        [0,  1,  0, -1, 1],  # 2  A AND NOT B
        [0,  1,  0,  0, 1],  # 3  A
        [0,  0,  1, -1, 1],  # 4  NOT A AND B
        [0,  0,  1,  0, 1],  # 5  B
        [0,  1,  1, -2, 1],  # 6  XOR
        [0,  1,  1, -1, 1],  # 7  OR
        [1, -1, -1,  1, 1],  # 8  NOR
        [1, -1, -1,  2, 1],  # 9  XNOR
        [1,  0, -1,  0, 1],  # 10 NOT B
        [1,  0, -1,  1, 1],  # 11 B -> A
        [1, -1,  0,  0, 1],  # 12 NOT A
        [1, -1,  0,  1, 1],  # 13 A -> B
        [1,  0,  0, -1, 1],  # 14 NAND
        [1,  0,  0,  0, 1],  # 15 TRUE
    ],
    dtype=np.float32,
)


def _split_waits(nc, maxw=1):
    """Walrus in this container encodes at most one sync-wait per
    instruction; hoist excess waits into preceding NoOps on the same
    engine (semantically an AND of waits, executed in sequence)."""
    for f in nc.m.functions:
        for blk in f.blocks:
            new_list = []
            changed = False
            for inst in blk.instructions:
                si = inst.sync_info
                if si is not None and len(si.on_wait) > maxw:
                    waits = list(si.on_wait)
                    chunks = [waits[i : i + maxw] for i in range(0, len(waits), maxw)]
                    for ci, ch in enumerate(chunks[:-1]):
                        nop = mybir.InstNoOp(
                            name=f"{inst.name}-wsplit{ci}", ins=[], outs=[]
                        )
                        nop.engine = inst.engine
                        nop.sync_info = mybir.SyncInfo(on_wait=ch, on_update=[])
                        new_list.append(nop)
                    inst.sync_info = mybir.SyncInfo(
                        on_wait=chunks[-1], on_update=list(si.on_update)
                    )
                    changed = True
                new_list.append(inst)
            if changed:
                blk.instructions = new_list


def build_nc(
    reps=1,
    mm_dt=None,
    wide=1,
    psum_bufs=3,
    probe=None,
    out16=False,
    epi16=False,
    # epilogue engine assignment: every `u_dve`-th u / `v_dve`-th v runs on
    # DVE instead of ACT; every `o_dve`-th o runs on DVE instead of GPSIMD.
    # 0 disables. Solved min-max balance: DVE=ACT=Pool at ~264us needs
    # u,v ~1/4 on DVE and o ~1/13 on DVE.
    u_dve=2,
    v_dve=0,
    o_dve=0,
    xbufs=2,
    # o = w + v as gpsimd scalar_tensor_tensor: REJECTED by walrus codegen
    # on the Pool engine (no Q7 kernel) -- keep False.
    o_stt=False,
):
    # reps>1 repeats the main loop inside the NEFF (timing only: slope
    # between two reps values isolates steady-state main-loop time from
    # the axon dispatch floor).
    # mm_dt: matmul dtype (F16 default; BF16 probe for PE-rate check —
    #   host packs the same dtype).
    # wide: nt-group size; x loaded as [128, wide*512] tiles and out
    #   accumulated as [128, wide*512] before DMA (wider DMA lines).
    # psum_bufs: main-loop PSUM pool depth (pa+pb pairs).
    F16_ = mm_dt or F16
    nc = bass.Bass()
    if USE_FP8_DR and PACK1:
        pk8_d = nc.dram_tensor(
            "pk8", [PREV, NB + WBYTES], mybir.dt.uint8, kind="ExternalInput"
        )
        x_d = pk8_d[:, 0:NB].bitcast(F8)
        w_d = pk8_d[:, NB:].bitcast(F16_)
        wa0, wb0, tw0 = 0, SIZE, 2 * SIZE
    elif USE_FP8_DR:
        x_d = nc.dram_tensor("xq", [PREV, NB], F8, kind="ExternalInput")
        w_d = nc.dram_tensor("wpk", [PREV, 2 * SIZE + 16], F16_, kind="ExternalInput")
        wa0, wb0, tw0 = 0, SIZE, 2 * SIZE
    elif USE_U8_X:
        x_d = nc.dram_tensor("xq", [PREV, NB], mybir.dt.uint8, kind="ExternalInput")
        w_d = nc.dram_tensor("wpk", [PREV, 2 * SIZE + 16], F16_, kind="ExternalInput")
        wa0, wb0, tw0 = 0, SIZE, 2 * SIZE
    else:
        pk_d = nc.dram_tensor("pk", [PREV, PKW], F16_, kind="ExternalInput")
        x_d, w_d = pk_d, pk_d
        wa0, wb0, tw0 = WA0, WB0, TW0
    BF16 = mybir.dt.bfloat16
    ODT = BF16 if out16 else F32
    EDT = BF16 if epi16 else F32
    out_d = nc.dram_tensor("out", [SIZE, NB], ODT, kind="ExternalOutput")
    c16_d = nc.inline_tensor(_C16, "c16")

    AF = mybir.ActivationFunctionType
    OP = mybir.AluOpType

    with tile.TileContext(nc) as tc:
        with (
            tc.tile_pool(name="persist", bufs=1) as pp,
            tc.tile_pool(name="wstage", bufs=3) as wstage,
            tc.tile_pool(name="xstage", bufs=6) as xstage,
            tc.tile_pool(name="xbuf", bufs=xbufs) as xbuf,
            tc.tile_pool(name="epi", bufs=3) as epi,
            tc.tile_pool(name="outp", bufs=2 if wide > 1 else 4) as outp,
        ):
            # Preamble PSUM pool is scoped: released before the main loop so
            # the main-loop pool can use up to all 8 banks.
            psp1_cm = tc.tile_pool(name="psum1", bufs=1, space="PSUM")
            psp1 = psp1_cm.__enter__()
            # --- constants ---
            c16s = pp.tile([16, 5], F32, tag="c16s", name="c16s")
            nc.sync.dma_start(out=c16s, in_=c16_d[:, :])
            c16h = pp.tile([16, 5], F16_, tag="c16h", name="c16h")
            nc.vector.tensor_copy(c16h, c16s)
            ones = pp.tile([128, 1], F16_, tag="ones", name="ones")
            nc.vector.memset(ones, 1.0)

            # --- table coefficients (tw stored transposed: [PREV, 16]) ---
            twt = pp.tile([16, SIZE], F16_, tag="twt", name="twt")
            if probe in ("notw", "nomain_notw"):
                nc.vector.memset(twt, 0.1)
            else:
                nc.sync.dma_start(
                    out=twt, in_=w_d[:, tw0 : tw0 + 16].rearrange("a b -> b a")
                )
            et = pp.tile([16, SIZE], F32, tag="et", name="et")
            nc.scalar.activation(et, twt, AF.Exp)
            # 16-bit PE matmuls carry limited precision; split et into
            # hi+lo halves and accumulate two exact matmuls.
            ethi = pp.tile([16, SIZE], F16_, tag="ethi", name="ethi")
            nc.vector.tensor_copy(ethi, et)
            etlo = pp.tile([16, SIZE], F16_, tag="etlo", name="etlo")
            nc.vector.scalar_tensor_tensor(
                etlo, et, 1.0, ethi, op0=OP.mult, op1=OP.subtract
            )
            psw = psp1.tile([128, MB, 5], F32, tag="psw", name="psw")
            for mb in range(MB):
                ms = slice(mb * 128, (mb + 1) * 128)
                nc.tensor.matmul(
                    psw[:, mb, :], ethi[:, ms], c16h[:, :], start=True, stop=False
                )
                nc.tensor.matmul(
                    psw[:, mb, :], etlo[:, ms], c16h[:, :], start=False, stop=True
                )

            # --- weights: exp in transposed layout + row sums ---
            # Row sums are over the QUANTIZED exps so the on-chip softmax is
            # an exact softmax of the quantized logits (common-mode error
            # cancels in the normalizer).
            pssa = psp1.tile([128, MB], F32, tag="pssa", name="pssa")
            pssb = psp1.tile([128, MB], F32, tag="pssb", name="pssb")
            DR = mybir.MatmulPerfMode.DoubleRow
            if USE_FP8_DR:
                NJ = KB // 2  # 4 k-pair blocks of 256
                eaT2 = [pp.tile([128, 2, SIZE], F8, tag=f"ea{j}", name=f"ea{j}") for j in range(NJ)]
                ebT2 = [pp.tile([128, 2, SIZE], F8, tag=f"eb{j}", name=f"eb{j}") for j in range(NJ)]
                ones2 = pp.tile([128, 2, 1], F8, tag="ones2", name="ones2")
                nc.vector.memset(ones2, 1.0)
                for j in range(NJ):
                    for i in range(2):
                        ks = slice(j * 256 + i * 128, j * 256 + (i + 1) * 128)
                        wfa = wstage.tile([128, SIZE], F16_, tag="wf16", name="wf16")
                        nc.sync.dma_start(out=wfa, in_=w_d[ks, wa0 : wa0 + SIZE])
                        nc.scalar.activation(eaT2[j][:, i, :], wfa, AF.Exp)
                        wfb = wstage.tile([128, SIZE], F16_, tag="wf16", name="wf16")
                        nc.sync.dma_start(out=wfb, in_=w_d[ks, wb0 : wb0 + SIZE])
                        nc.scalar.activation(ebT2[j][:, i, :], wfb, AF.Exp)
                for mb in range(MB):
                    ms = slice(mb * 128, (mb + 1) * 128)
                    for j in range(NJ):
                        nc.tensor.matmul(
                            pssa[:, mb : mb + 1],
                            eaT2[j][:, :, ms],
                            ones2[:, :, :],
                            start=(j == 0),
                            stop=(j == NJ - 1),
                            perf_mode=DR,
                        )
                    for j in range(NJ):
                        nc.tensor.matmul(
                            pssb[:, mb : mb + 1],
                            ebT2[j][:, :, ms],
                            ones2[:, :, :],
                            start=(j == 0),
                            stop=(j == NJ - 1),
                            perf_mode=DR,
                        )
            else:
                eaT = [pp.tile([128, SIZE], F16_, tag=f"ea{kb}", name=f"ea{kb}") for kb in range(KB)]
                ebT = [pp.tile([128, SIZE], F16_, tag=f"eb{kb}", name=f"eb{kb}") for kb in range(KB)]
                for kb in range(KB):
                    ks = slice(kb * 128, (kb + 1) * 128)
                    wfa = wstage.tile([128, SIZE], F16_, tag="wf16", name="wf16")
                    nc.sync.dma_start(out=wfa, in_=w_d[ks, wa0 : wa0 + SIZE])
                    nc.scalar.activation(eaT[kb], wfa, AF.Exp)
                    wfb = wstage.tile([128, SIZE], F16_, tag="wf16", name="wf16")
                    nc.sync.dma_start(out=wfb, in_=w_d[ks, wb0 : wb0 + SIZE])
                    nc.scalar.activation(ebT[kb], wfb, AF.Exp)
                # mb-outer so each column's PSUM accumulation group is
                # contiguous in PE order — interleaved groups in one bank
                # corrupt results.
                for mb in range(MB):
                    ms = slice(mb * 128, (mb + 1) * 128)
                    for kb in range(KB):
                        nc.tensor.matmul(
                            pssa[:, mb : mb + 1],
                            eaT[kb][:, ms],
                            ones[:, :],
                            start=(kb == 0),
                            stop=(kb == KB - 1),
                        )
                    for kb in range(KB):
                        nc.tensor.matmul(
                            pssb[:, mb : mb + 1],
                            ebT[kb][:, ms],
                            ones[:, :],
                            start=(kb == 0),
                            stop=(kb == KB - 1),
                        )

            # --- assemble final coefficients [128, MB] ---
            sa = pp.tile([128, MB], F32, tag="sa", name="sa")
            nc.vector.tensor_copy(sa, pssa)
            sb = pp.tile([128, MB], F32, tag="sb", name="sb")
            nc.vector.tensor_copy(sb, pssb)
            ra = pp.tile([128, MB], F32, tag="ra", name="ra")
            nc.vector.reciprocal(ra, sa)
            rb = pp.tile([128, MB], F32, tag="rb", name="rb")
            nc.vector.reciprocal(rb, sb)
            wraw = pp.tile([128, MB, 5], F32, tag="wraw", name="wraw")
            nc.vector.tensor_copy(wraw, psw)
            rt = pp.tile([128, MB], F32, tag="rt", name="rt")
            nc.vector.reciprocal(rt, wraw[:, :, 4])
            tA = pp.tile([128, MB], F32, tag="tA", name="tA")
            nc.vector.tensor_mul(tA, rt, ra)
            tB = pp.tile([128, MB], F32, tag="tB", name="tB")
            nc.vector.tensor_mul(tB, rt, rb)
            tAB = pp.tile([128, MB], F32, tag="tAB", name="tAB")
            nc.vector.tensor_mul(tAB, tA, rb)
            w1f = pp.tile([128, MB], F32, tag="w1f", name="w1f")
            nc.vector.tensor_mul(w1f, wraw[:, :, 0], rt)
            wAf = pp.tile([128, MB], F32, tag="wAf", name="wAf")
            nc.vector.tensor_mul(wAf, wraw[:, :, 1], tA)
            wBf = pp.tile([128, MB], F32, tag="wBf", name="wBf")
            nc.vector.tensor_mul(wBf, wraw[:, :, 2], tB)
            wABf = pp.tile([128, MB], F32, tag="wABf", name="wABf")
            nc.vector.tensor_mul(wABf, wraw[:, :, 3], tAB)

            psp1_cm.__exit__(None, None, None)
            psp_cm = tc.tile_pool(name="psum", bufs=psum_bufs, space="PSUM")
            psp = psp_cm.__enter__()

            # --- main loop ---
            # probe (timing-ablation builds; not for real results):
            #   'noepi' - x DMA + matmuls only (tiny DVE consumer)
            #   'noout' - no output DMA
            #   'nox'   - constant x (no x DMA)
            GW = wide * NT
            NG = N_NT // wide
            if probe in ("nomain", "nomain_notw"):
                reps = 0
            xconst = None
            if probe == "nox":
                xconst = [
                    pp.tile([128, GW], F16_, tag=f"xc{kb}", name=f"xc{kb}")
                    for kb in range(KB)
                ]
                for t in xconst:
                    nc.vector.memset(t, 0.25)
            for _rep in range(reps):
              for ntg in range(NG):
                gs = slice(ntg * GW, (ntg + 1) * GW)
                xb = []
                if USE_FP8_DR and probe != "nox":
                    # one DMA instruction for all 8 k-blocks of this group
                    xall = xbuf.tile([128, KB, GW], F8, tag="xall", name="xall")
                    nc.sync.dma_start(
                        out=xall,
                        in_=x_d[:, gs].rearrange("(c p) n -> p c n", p=128),
                    )
                    xb = xall
                else:
                  for kb in range(KB):
                    ks = slice(kb * 128, (kb + 1) * 128)
                    if probe == "nox":
                        xb.append(xconst[kb])
                        continue
                    if USE_U8_X:
                        xq = xstage.tile([128, GW], mybir.dt.uint8, tag="xq", name="xq")
                        nc.sync.dma_start(out=xq, in_=x_d[ks, gs])
                        xbt = xbuf.tile([128, GW], F16_, tag=f"xb{kb}", name=f"xb{kb}")
                        # dequant: x = (k + 0.5) / 256
                        nc.scalar.activation(
                            xbt, xq, AF.Copy, bias=1.0 / 512, scale=1.0 / 256
                        )
                    else:
                        xbt = xbuf.tile([128, GW], F16_, tag=f"xb{kb}", name=f"xb{kb}")
                        nc.sync.dma_start(out=xbt, in_=x_d[ks, gs])
                    xb.append(xbt)
                otall = None
                if USE_FP8_DR:
                    otall = outp.tile([128, MB, GW], ODT, tag="otall", name="otall")
                for mb in range(MB):
                    ms = slice(mb * 128, (mb + 1) * 128)
                    if USE_FP8_DR:
                        ot = otall[:, mb, :]
                    else:
                        ot = outp.tile([128, GW], F32, tag=f"o{mb}", name=f"o{mb}")
                    for w_off in range(wide):
                        ws_ = slice(w_off * NT, (w_off + 1) * NT)
                        pa = psp.tile([128, NT], F32, tag="pa", name="pa")
                        pb = psp.tile([128, NT], F32, tag="pb", name="pb")
                        if USE_FP8_DR:
                            NJ = KB // 2
                            for j in range(NJ):
                                nc.tensor.matmul(
                                    pa,
                                    eaT2[j][:, :, ms],
                                    xb[:, 2 * j : 2 * j + 2, ws_],
                                    start=(j == 0),
                                    stop=(j == NJ - 1),
                                    perf_mode=DR,
                                )
                            for j in range(NJ):
                                nc.tensor.matmul(
                                    pb,
                                    ebT2[j][:, :, ms],
                                    xb[:, 2 * j : 2 * j + 2, ws_],
                                    start=(j == 0),
                                    stop=(j == NJ - 1),
                                    perf_mode=DR,
                                )
                        else:
                          for kb in range(KB):
                            nc.tensor.matmul(
                                pa,
                                eaT[kb][:, ms],
                                xb[kb][:, ws_],
                                start=(kb == 0),
                                stop=(kb == KB - 1),
                            )
                          for kb in range(KB):
                            nc.tensor.matmul(
                                pb,
                                ebT[kb][:, ms],
                                xb[kb][:, ws_],
                                start=(kb == 0),
                                stop=(kb == KB - 1),
                            )
                        if probe == "noepi":
                            s = epi.tile([128, 16], F32, tag="s", name="s")
                            nc.vector.tensor_copy(s[:, 0:8], pa[:, 0:8])
                            nc.vector.tensor_copy(s[:, 8:16], pb[:, 0:8])
                            continue
                        # epilogue:
                        #   u = tb*wAB' + wA'      (DVE / ACT, alternating --
                        #                           both are scale*in+bias;
                        #                           balances engine load)
                        #   v = tb*wB' + w1'       (ACT identity scale/bias)
                        #   w = ta*u               (DVE)
                        #   o = w + v              (GPSIMD, SBUF only)
                        idx = ntg * MB + mb
                        u = epi.tile([128, NT], EDT, tag="u", name="u")
                        if u_dve and idx % u_dve == 0:
                            nc.vector.tensor_scalar(
                                u,
                                pb,
                                wABf[:, mb : mb + 1],
                                wAf[:, mb : mb + 1],
                                op0=OP.mult,
                                op1=OP.add,
                            )
                        else:
                            nc.scalar.activation(
                                u,
                                pb,
                                AF.Identity,
                                bias=wAf[:, mb : mb + 1],
                                scale=wABf[:, mb : mb + 1],
                            )
                        v = epi.tile([128, NT], EDT, tag="v", name="v")
                        if v_dve and idx % v_dve == 1:
                            nc.vector.tensor_scalar(
                                v,
                                pb,
                                wBf[:, mb : mb + 1],
                                w1f[:, mb : mb + 1],
                                op0=OP.mult,
                                op1=OP.add,
                            )
                        else:
                            nc.scalar.activation(
                                v,
                                pb,
                                AF.Identity,
                                bias=w1f[:, mb : mb + 1],
                                scale=wBf[:, mb : mb + 1],
                            )
                        w = epi.tile([128, NT], F32, tag="w", name="w")
                        nc.vector.tensor_mul(w, pa, u)
                        if o_dve and idx % o_dve == 2:
                            nc.vector.tensor_add(ot[:, ws_], w, v)
                        elif o_stt:
                            nc.gpsimd.scalar_tensor_tensor(
                                ot[:, ws_], w, 0.0, v, op0=OP.add, op1=OP.add
                            )
                        else:
                            nc.gpsimd.tensor_add(ot[:, ws_], w, v)
                    if probe not in ("noepi", "noout") and not USE_FP8_DR:
                        nc.sync.dma_start(out=out_d[ms, gs], in_=ot)
                if USE_FP8_DR and probe not in ("noepi", "noout"):
                    nc.sync.dma_start(
                        out=out_d[:, gs].rearrange("(m p) n -> p m n", p=128),
                        in_=otall,
                    )

            psp_cm.__exit__(None, None, None)

    _split_waits(nc)
    return nc


_NC_CACHE = None
_EXEC_CACHE = None


def _get_nc():
    global _NC_CACHE
    if _NC_CACHE is None:
        _NC_CACHE = build_nc()
    return _NC_CACHE


def make_exec(nc):
    """Single-core executable for `nc`: inputs only (no zero-output staging
    operands — the kernel writes every output element), plain jit on
    device 0."""
    import jax
    import concourse.bass2jax as b2j

    b2j.install_neuronx_cc_hook()

    part_name = nc.partition_id_tensor.name if nc.partition_id_tensor else None
    in_names, out_names, out_avals = [], [], []
    for alloc in nc.m.functions[0].allocations:
        if not isinstance(alloc, mybir.MemoryLocationSet):
            continue
        name = alloc.memorylocations[0].name
        if alloc.kind == "ExternalInput":
            if name != part_name:
                in_names.append(name)
        elif alloc.kind == "ExternalOutput":
            out_names.append(name)
            out_avals.append(
                jax.core.ShapedArray(
                    tuple(alloc.tensor_shape), mybir.dt.np(alloc.dtype)
                )
            )
    all_in_names = list(in_names)
    if part_name is not None:
        all_in_names.append(part_name)

    def _body(*args):
        operands = list(args)
        if part_name is not None:
            operands.append(b2j.partition_id_tensor())
        outs = b2j._bass_exec_p.bind(
            *operands,
            out_avals=tuple(out_avals),
            in_names=tuple(all_in_names),
            out_names=tuple(out_names),
            lowering_input_output_aliases=(),
            sim_require_finite=True,
            sim_require_nnan=True,
            nc=nc,
        )
        return tuple(outs)

    return jax.jit(_body), in_names


def pack_inputs(
    prev_layer_output,
    input_A_weights,
    input_B_weights,
    table_weights,
    mm_dt=None,
):
    npdt = mybir.dt.np(mm_dt or F16)
    x = np.asarray(prev_layer_output, dtype=np.float32)
    wa = np.asarray(input_A_weights, dtype=np.float32)
    wb = np.asarray(input_B_weights, dtype=np.float32)
    tw = np.asarray(table_weights, dtype=np.float32)
    if USE_FP8_DR:
        xq = np.ascontiguousarray(x.astype(mybir.dt.np(F8)))
        wpk = np.empty((PREV, 2 * SIZE + 16), npdt)
        wpk[:, 0:SIZE] = wa.T
        wpk[:, SIZE : 2 * SIZE] = wb.T
        wpk[:, 2 * SIZE :] = tw.T
        if PACK1:
            pk8 = np.empty((PREV, NB + WBYTES), np.uint8)
            pk8[:, :NB] = xq.view(np.uint8)
            pk8[:, NB:] = np.ascontiguousarray(wpk).view(np.uint8)
            return {"pk8": pk8}
        return {"xq": xq, "wpk": wpk}
    if USE_U8_X:
        xq = np.clip(np.floor(x * 256.0), 0, 255).astype(np.uint8)
        wpk = np.empty((PREV, 2 * SIZE + 16), npdt)
        wpk[:, 0:SIZE] = wa.T
        wpk[:, SIZE : 2 * SIZE] = wb.T
        wpk[:, 2 * SIZE :] = tw.T
        return {"xq": np.ascontiguousarray(xq), "wpk": wpk}
    pk = np.empty((PREV, PKW), npdt)
    pk[:, XC0:WA0] = x
    pk[:, WA0:WB0] = wa.T
    pk[:, WB0:TW0] = wb.T
    pk[:, TW0:PKW] = tw.T
    return {"pk": pk}


def kernel(prev_layer_output, input_A_weights, input_B_weights, table_weights):
    global _EXEC_CACHE
    import jax

    if _EXEC_CACHE is None:
        _EXEC_CACHE = make_exec(_get_nc())
    run, in_names = _EXEC_CACHE

    inp = pack_inputs(
        prev_layer_output, input_A_weights, input_B_weights, table_weights
    )
    dev = jax.devices()[0]
    args = [jax.device_put(inp[nm], dev) for nm in in_names]
    out = run(*args)
    return np.asarray(out[0])
